# revision 21
# baseline (speedup 1.0000x reference)
"""GCN (3-layer, PyG GCNConv-style) forward pass on 8 Trainium2 NeuronCores.

Architecture v6 (L1 direct from replicated X-table; no AG-L1):
  - Aggregation commutes with the GEMM: segsum(X~[src]) @ W1 ==
    segsum((X~ W1)[src]) where X~ = dis*x.  Every core receives the full
    x, so each core holds the complete dis-prescaled X table from t=0 and
    L1 edge-gathers start immediately -- layer 1 needs NO AllGather and
    no Z1 pre-GEMM phase.
  - L1 per dst tile (feature-major): SWDGE dma_gather of unique (src,slot)
    rows from the X table + fp8-selector matmuls accumulate
    A1fm [256 x 128dst] in PSUM (self-loop via identity matmul on a
    node-major x tile); then W1 GEMM, dis/bias/relu post, W2 GEMM and a
    single PE transpose produce node-major zb2 for the L2 table.
  - Each layer AllGathers its Z table in two chunks -- hot (tiles <
    JCUT, feeds the PE-scatter) and cold -- fired via per-chunk counters;
    L1 processes hot dst tiles first so the hot AG dispatches ~halfway
    through L1 and the 88-step scatter fully overlaps the cold gathers.
  - Layers 2/3 keep the hot/cold split: hot source tiles via PE-scatter
    (Z stationary, 0/1 fp8 adjacency slices streamed as moving operand
    into a feature-major PSUM [d x 2560] in five 512-col banks), cold
    source tiles via SWDGE gather + per-dst-tile selector matmuls.
  - The layer tail is pipelined per 512-col chunk: close chunk c (cold
    matmuls) -> feature-major post -> next-layer GEMM -> AllGather chunk
    fires as soon as its tiles are stored.  Final output is PE-transposed
    back to node-major.
"""

import sys

import numpy as np

sys.path.insert(0, "/opt/trn_rl_repo")

import ml_dtypes  # noqa: E402

import concourse.bass as bass  # noqa: E402
import concourse.bacc as bacc  # noqa: E402
import concourse.mybir as mybir  # noqa: E402
from concourse.bass_utils import run_bass_kernel_spmd  # noqa: E402
from concourse.library_config import mlp as _mlp_lib  # noqa: E402
from concourse.tile import TileContext  # noqa: E402
from concourse.tile_rust import add_dep_helper  # noqa: E402

BF16 = ml_dtypes.bfloat16
FP8 = ml_dtypes.float8_e4m3

# ----------------------------------------------------------------------------
# Problem configuration (hardcoded for nn_Encoder_17386027614431)
# ----------------------------------------------------------------------------
N_NODES = 20000
N_CORES = 8
T = 128
NT = 20                  # dst tiles per core
SHARD = NT * T           # 2560
NTAB = N_CORES * SHARD   # 20480 table rows
D0 = 256
DL = [256, 128, 64]      # per-layer output dims
HT_A = 11                # tiles in AllGather chunk a (= all hot tiles)
JCUT = 11                # L2: tiles >= JCUT are cold (edges via gather)
JCUT3 = 11               # L3: tiles >= JCUT3 are cold
JC = {1: JCUT, 2: JCUT3}
SA_TILES = N_CORES * HT_A          # 32 src tiles in chunk-a table
HOTB = JCUT - HT_A                 # hot b1 tiles per core in smat layout
B1N = {1: JCUT - HT_A, 2: JCUT3 - HT_A}   # b1 tiles per core per layer
DPAD = [256, 128, 128]             # table row widths (L3 padded)
CCHUNK = 512                       # psum bank columns (f32)
NCH = SHARD // CCHUNK              # 5 feature-major column chunks
# AG chunk tile ranges per layer (index 0 unused in v6): one hot chunk
# (feeds the PE-scatter) + one cold chunk (feeds the cold gathers).
AGCH = [
    None,
    [(0, JCUT), (JCUT, NT)],
    [(0, JCUT3), (JCUT3, NT)],
]
# L1 dst-tile processing order: natural -- hot tiles (0..JCUT) complete
# first so the hot AG dispatches ~halfway through L1, then the cold tiles.
TILE_ORDER = list(range(NT))


def _ru16(x):
    return (int(x) + 15) // 16 * 16


def _offsets(cnt2d):
    flat = [c for row in cnt2d for c in row]
    nbl = [(c + T - 1) // T for c in flat]
    boff, ioff = [], []
    ob = oi = 0
    for c, nb in zip(flat, nbl):
        boff.append(ob)
        ioff.append(oi)
        ob += nb
        oi += c // 16
    return nbl, boff, ioff, ob, oi


def _build_nc(CNT, CNTC, apply_b1, apply_b2, apply_b3):
    f32 = mybir.dt.float32
    bf16 = mybir.dt.bfloat16
    fp8 = mybir.dt.float8e4
    i16 = mybir.dt.int16
    mult = mybir.AluOpType.mult
    add = mybir.AluOpType.add
    relu = mybir.ActivationFunctionType.Relu
    fcopy = mybir.ActivationFunctionType.Copy

    nbl, boff, ioff, totblk, idxcols = _offsets([CNT])
    nblc, boffc, ioffc, totblkc, idxcolsc = {}, {}, {}, {}, {}
    for l in (1, 2):
        nblc[l], boffc[l], ioffc[l], totblkc[l], idxcolsc[l] = _offsets(
            [CNTC[l]])
    maxnb = max(nbl)
    maxnbc = max(max(nblc[1]), max(nblc[2]))

    nc = bacc.Bacc("TRN2", num_devices=N_CORES, num_swdge_queues=4)

    # ---- kernel I/O ----
    xtab = nc.dram_tensor("xtab", [NTAB, D0], bf16, kind="ExternalInput")
    xnm = nc.dram_tensor("xnm", [T, NT * D0], bf16, kind="ExternalInput")
    w1 = nc.dram_tensor("w1", [D0, DL[0]], bf16, kind="ExternalInput")
    w2 = nc.dram_tensor("w2", [DL[0], DL[1]], bf16, kind="ExternalInput")
    w3 = nc.dram_tensor("w3", [DL[1], DL[2]], bf16, kind="ExternalInput")
    b1c2 = nc.dram_tensor("b1c2", [T, 2], f32, kind="ExternalInput")
    b2col = nc.dram_tensor("b2col", [T, 1], f32, kind="ExternalInput")
    b3col = nc.dram_tensor("b3col", [T, 1], f32, kind="ExternalInput")
    dis = nc.dram_tensor("dis", [T, NT], f32, kind="ExternalInput")
    disrow = nc.dram_tensor("disrow", [T, SHARD], bf16,
                            kind="ExternalInput")
    disrow2 = nc.dram_tensor("disrow2", [T, SHARD], bf16,
                             kind="ExternalInput")
    identb = nc.dram_tensor("identb", [T, T], bf16, kind="ExternalInput")
    identf = nc.dram_tensor("identf", [T, T], f32, kind="ExternalInput")
    idx = nc.dram_tensor("idx", [T, idxcols], i16, kind="ExternalInput")
    sel = nc.dram_tensor("sel", [T, totblk * T], fp8, kind="ExternalInput")
    idxc = {l: nc.dram_tensor(f"idxc{l}", [T, idxcolsc[l]], i16,
                              kind="ExternalInput") for l in (1, 2)}
    selc = {l: nc.dram_tensor(f"selc{l}", [T, totblkc[l] * T], fp8,
                              kind="ExternalInput") for l in (1, 2)}
    smat = nc.dram_tensor("smat", [NTAB, SHARD], fp8, kind="ExternalInput")
    out = nc.dram_tensor("out", [SHARD, DL[2]], f32, kind="ExternalOutput")

    # ---- internal DRAM for collectives (per layer, per AG chunk) ----
    agin = {l: [] for l in (1, 2)}
    agout = {l: [] for l in (1, 2)}
    for l in (1, 2):
        for k, (j0, j1) in enumerate(AGCH[l]):
            rows = (j1 - j0) * T
            agin[l].append(nc.dram_tensor(
                f"agin{l}_{k}", [rows, DPAD[l]], bf16))
            agout[l].append(nc.dram_tensor(
                f"agout{l}_{k}", [N_CORES * rows, DPAD[l]], bf16,
                addr_space="Shared"))
    rg = [list(range(N_CORES))]

    with TileContext(nc) as tc:
        nc.gpsimd.load_library(_mlp_lib)

        with (
            tc.tile_pool(name="const", bufs=1) as cpool,
            tc.tile_pool(name="sb", bufs=12) as sbpool,        # S stream
            tc.tile_pool(name="zsb", bufs=3) as zspool,       # Z stationary
            tc.tile_pool(name="selp", bufs=3) as selpool,
            tc.tile_pool(name="hp", bufs=2) as hpool,
            tc.tile_pool(name="htp", bufs=3) as htpool,
            tc.tile_pool(name="tmp", bufs=3) as tpool,
            tc.tile_pool(name="zbp", bufs=3) as zbpool,
            tc.tile_pool(name="ps_z", bufs=1, space="PSUM") as ps_z,
            tc.tile_pool(name="ps_agg", bufs=1, space="PSUM") as ps_agg,
            tc.tile_pool(name="ps_t", bufs=1, space="PSUM") as ps_t,
            tc.tile_pool(name="ps_fm", bufs=1, space="PSUM") as ps_fm,
        ):
            # ---- constants (idx/sel path first so gathers start at t=0) ----
            def load_const(dram_h, shape, dtype):
                t = cpool.tile(shape, dtype, tag=f"c_{dram_h.name}")
                nc.sync.dma_start(out=t[:, :], in_=dram_h.ap())
                return t

            def load_const_chunked(dram_h, inner, dtype):
                cs = dram_h.shape[0] // T
                t = cpool.tile([T, cs * inner], dtype, tag=f"c_{dram_h.name}")
                nc.sync.dma_start(
                    out=t.rearrange("p (c n) -> p c n", c=cs),
                    in_=dram_h.ap().rearrange("(c p) n -> p c n", p=T),
                )
                return t

            idx_sb = load_const(idx, [T, idxcols], i16)
            xnm_sb = load_const(xnm, [T, NT * D0], bf16)
            w1_sb = load_const_chunked(w1, DL[0], bf16)
            identb_sb = load_const(identb, [T, T], bf16)
            disrow_sb = load_const(disrow, [T, SHARD], bf16)
            disrow2_sb = load_const(disrow2, [T, SHARD], bf16)
            b1c2_sb = load_const(b1c2, [T, 2], f32)
            idxc_sb = {l: load_const(idxc[l], [T, idxcolsc[l]], i16)
                       for l in (1, 2)}
            w2_sb = load_const_chunked(w2, DL[1], bf16)
            w3_sb = load_const(w3, [DL[1], DL[2]], bf16)
            dis_sb = load_const(dis, [T, NT], f32)
            b2_sb = load_const(b2col, [T, 1], f32)
            b3_sb = load_const(b3col, [T, 1], f32)
            identf_sb = load_const(identf, [T, T], f32)

            # persistent buffers
            gbuf = [cpool.tile([T, maxnb * D0], bf16, tag=f"g{i}",
                               name=f"gbuf{i}") for i in range(3)]
            for g in gbuf:
                nc.vector.memset(g[:, :], 0.0)
            h2fm = cpool.tile([T, SHARD], bf16, tag="h2fm")
            outfm = cpool.tile([T, SHARD], f32, tag="outfm")
            fm = [ps_fm.tile([T, CCHUNK], f32, tag=f"fm{c}",
                             name=f"fm{c}") for c in range(NCH)]
            cbuf = [cpool.tile([T, maxnbc * DPAD[1]], bf16, tag=f"cb{j}",
                               name=f"cbuf{j}") for j in range(NT)]
            for cb in cbuf:
                nc.vector.memset(cb[:, :], 0.0)

            agin_v = {l: [agin[l][k].ap().rearrange("(n p) d -> p n d", p=T)
                          for k in range(len(AGCH[l]))] for l in (1, 2)}
            agout_v = {l: [agout[l][k].ap().rearrange("(n p) d -> p n d", p=T)
                           for k in range(len(AGCH[l]))] for l in (1, 2)}
            smat_v = smat.ap().rearrange("(s p) d -> p s d", p=T)
            out_v = out.ap().rearrange("(n p) d -> p n d", p=T)

            ag_insts = {l: [None] * len(AGCH[l]) for l in (1, 2)}
            agin_dmas = {l: [[] for _ in AGCH[l]] for l in (1, 2)}
            ag_left = {l: [j1 - j0 for (j0, j1) in AGCH[l]] for l in (1, 2)}

            def issue_ag(l, k):
                cc = nc.gpsimd.collective_compute(
                    "AllGather",
                    mybir.AluOpType.bypass,
                    replica_groups=rg,
                    ins=[agin[l][k].ap().opt()],
                    outs=[agout[l][k].ap().opt()],
                )
                for d in agin_dmas[l][k]:
                    add_dep_helper(cc.ins, d.ins, reason=f"ag{l}.{k}")
                ag_insts[l][k] = cc

            def z_store(l, j, zb):
                for k, (j0, j1) in enumerate(AGCH[l]):
                    if j0 <= j < j1:
                        break
                d = nc.sync.dma_start(
                    out=agin_v[l][k][:, j - j0, :], in_=zb[:, :])
                agin_dmas[l][k].append(d)
                ag_left[l][k] -= 1
                if ag_left[l][k] == 0:
                    issue_ag(l, k)

            # ---- cold gathers / matmuls for L2/L3 ----
            gq = [0]

            def cold_gathers(l):
                for j in range(NT):
                    cnt = CNTC[l][j]
                    gt3 = cbuf[j][:, :nblc[l][j] * DPAD[l]].rearrange(
                        "p (n d) -> p n d", d=DPAD[l])
                    g = nc.gpsimd.dma_gather(
                        gt3,
                        agout[l][1].ap(),
                        idxc_sb[l][:, ioffc[l][j]:ioffc[l][j] + cnt // 16],
                        cnt, cnt, DPAD[l],
                        single_packet=False,
                        queue_num=gq[0] % 4,
                    )
                    gq[0] += 1
                    add_dep_helper(g.ins, ag_insts[l][1].ins, reason="cg ag")

            def cold_matmuls_chunk(l, c):
                """Fold cold edges of dst tiles 4c..4c+3 into fm[c]; the last
                one closes the accumulation group."""
                d_el = DL[l]
                for j in range(4 * c, 4 * c + 4):
                    nb = nblc[l][j]
                    gt3 = cbuf[j][:, :nb * DPAD[l]].rearrange(
                        "p (n d) -> p n d", d=DPAD[l])
                    st = selpool.tile([T, max(maxnb, maxnbc) * T], fp8,
                                      tag="sel")
                    nc.sync.dma_start(
                        out=st[:, :nb * T],
                        in_=selc[l][:, boffc[l][j] * T:
                                   (boffc[l][j] + nb) * T])
                    r = (j % 4) * T
                    for b in range(nb):
                        nc.tensor.matmul(
                            fm[c][:d_el, r:r + T],
                            gt3[:, b, :d_el],
                            st[:, b * T:(b + 1) * T],
                            start=False,
                            stop=(j % 4 == 3 and b == nb - 1),
                            skip_group_check=True)

            # ---- hot scatter: chunk-a tiles then per-core b1 stripes; S
            # fetched two tiles per DMA on the Activation HWDGE queue. ----
            fetch_plan = {}           # per layer: (smat_row0, ntiles)
            hot_steps = {}
            for l in (1, 2):
                fp_ = []
                for g2 in range(SA_TILES // 2):
                    fp_.append((2 * g2, 2))
                for core in range(N_CORES):
                    base = SA_TILES + core * HOTB
                    k = 0
                    while k < B1N[l]:
                        n = min(2, B1N[l] - k)
                        fp_.append((base + k, n))
                        k += n
                fetch_plan[l] = fp_
                hs = []
                for fi, (r0, n) in enumerate(fp_):
                    for k in range(n):
                        hs.append((fi, k))
                hot_steps[l] = hs
            n_hot = {l: len(hot_steps[l]) for l in (1, 2)}

            scat = {"pos": 0, "zsb": None, "stile": None}

            def scatter_steps(l, n, limit):
                d_el = DL[l]
                dp = DPAD[l]
                b1n = B1N[l]
                while n > 0 and scat["pos"] < limit:
                    pos = scat["pos"]
                    fi, k = hot_steps[l][pos]
                    r0, fn = fetch_plan[l][fi]
                    if pos < SA_TILES:
                        if pos % 8 == 0:
                            zsb = zspool.tile([T, 8 * dp], bf16,
                                              tag=f"zsa{l}")
                            d = nc.sync.dma_start(
                                out=zsb.rearrange("p (n d) -> p n d", d=dp),
                                in_=agout_v[l][0][:, pos:pos + 8, :])
                            add_dep_helper(d.ins, ag_insts[l][0].ins,
                                           reason="zs ag")
                            scat["zsb"] = zsb
                        zk = pos % 8
                    else:
                        p = pos - SA_TILES
                        if p % b1n == 0:
                            core = p // b1n
                            zsb = zspool.tile([T, b1n * dp], bf16,
                                              tag=f"zsb{l}")
                            d = nc.sync.dma_start(
                                out=zsb.rearrange("p (n d) -> p n d", d=dp),
                                in_=agout_v[l][1][:, core * b1n:
                                                  (core + 1) * b1n, :])
                            add_dep_helper(d.ins, ag_insts[l][1].ins,
                                           reason="zs ag")
                            scat["zsb"] = zsb
                        zk = p % b1n
                    if k == 0:
                        stile = sbpool.tile([T, 2 * SHARD], fp8, tag="sm")
                        nc.scalar.dma_start(
                            out=stile[:, :fn * SHARD].rearrange(
                                "p (n d) -> p n d", d=SHARD),
                            in_=smat_v[:, r0:r0 + fn, :])
                        scat["stile"] = stile
                    stile = scat["stile"]
                    zsb = scat["zsb"]
                    for c in range(NCH):
                        nc.tensor.matmul(
                            fm[c][:d_el, :],
                            zsb[:, zk * dp:zk * dp + d_el],
                            stile[:, k * SHARD + c * CCHUNK:
                                  k * SHARD + (c + 1) * CCHUNK],
                            start=(pos == 0), stop=False,
                            skip_group_check=True)
                    scat["pos"] = pos + 1
                    n -= 1

            # ================= Layer 1 (from the replicated X table) =======
            for ji, j in enumerate(TILE_ORDER):
                cnt = CNT[j]
                nb = nbl[j]
                gslot = ji % 3
                gt3 = gbuf[gslot][:, :nb * D0].rearrange(
                    "p (n d) -> p n d", d=D0)
                nc.gpsimd.dma_gather(
                    gt3,
                    xtab.ap(),
                    idx_sb[:, ioff[j]:ioff[j] + cnt // 16],
                    cnt, cnt, D0,
                    single_packet=False,
                    queue_num=gq[0] % 4,
                )
                gq[0] += 1
                st = selpool.tile([T, max(maxnb, maxnbc) * T], fp8, tag="sel")
                nc.sync.dma_start(
                    out=st[:, :nb * T],
                    in_=sel[:, boff[j] * T:(boff[j] + nb) * T])
                # A1fm [256feat x 128dst] in two halves; self-loop first
                a1 = ps_agg.tile([T, 2 * T], f32, tag="agg")
                for h in range(2):
                    nc.tensor.matmul(
                        a1[:, h * T:(h + 1) * T],
                        xnm_sb[:, j * D0 + h * T:j * D0 + (h + 1) * T],
                        identb_sb[:, :],
                        start=True, stop=False)
                    for b in range(nb):
                        nc.tensor.matmul(
                            a1[:, h * T:(h + 1) * T],
                            gt3[:, b, h * T:(h + 1) * T],
                            st[:, b * T:(b + 1) * T],
                            start=False, stop=(b == nb - 1))
                a1sb = tpool.tile([T, 2 * T], bf16, tag="a1")
                nc.scalar.activation(a1sb[:, :], a1[:, :], fcopy)
                # h1fm = relu(dis^2 * (A1 @ W1)) (or dis/b1 variant)
                zp = ps_z.tile([T, 2 * DL[0]], f32, tag="zp")
                for h in range(2):
                    for cch in range(2):
                        nc.tensor.matmul(
                            zp[:, h * T:(h + 1) * T],
                            w1_sb[:, cch * D0 + h * T:cch * D0 + (h + 1) * T],
                            a1sb[:, cch * T:(cch + 1) * T],
                            start=(cch == 0), stop=(cch == 1))
                t1 = tpool.tile([T, 2 * T], f32, tag="post")
                drow = disrow_sb if apply_b1 else disrow2_sb
                for h in range(2):
                    nc.vector.tensor_tensor(
                        t1[:, h * T:(h + 1) * T], zp[:, h * T:(h + 1) * T],
                        drow[:, j * T:(j + 1) * T], mult)
                h1sb = hpool.tile([T, 2 * T], bf16, tag="h1")
                if apply_b1:
                    for h in range(2):
                        nc.scalar.activation(
                            h1sb[:, h * T:(h + 1) * T],
                            t1[:, h * T:(h + 1) * T], relu,
                            bias=b1c2_sb[:, h:h + 1])
                else:
                    nc.scalar.activation(h1sb[:, :], t1[:, :], relu)
                # zb2fm = (h1 @ W2)^T then one transpose to node-major
                for cch in range(2):
                    nc.tensor.matmul(
                        zp[:, 2 * T:2 * T + DL[1]],
                        w2_sb[:, cch * DL[1]:(cch + 1) * DL[1]],
                        h1sb[:, cch * T:(cch + 1) * T],
                        start=(cch == 0), stop=(cch == 1))
                zfs = zbpool.tile([T, DL[1]], bf16, tag="z2f")
                if apply_b1:
                    nc.vector.tensor_tensor(
                        zfs[:, :], zp[:, 2 * T:2 * T + DL[1]],
                        disrow_sb[:, j * T:(j + 1) * T], mult)
                else:
                    nc.scalar.activation(
                        zfs[:, :], zp[:, 2 * T:2 * T + DL[1]], fcopy)
                tp = ps_t.tile([T, T], bf16, tag="tp")
                nc.tensor.matmul(tp[:, :], zfs[:, :], identb_sb[:, :],
                                 is_transpose=True)
                zb2 = zbpool.tile([T, DL[1]], bf16, tag="zb2")
                nc.scalar.activation(zb2[:, :], tp[:, :], fcopy)
                z_store(1, j, zb2)

            # ---- layer tails: finish aggregation, pipeline per chunk ----
            def layer_tail(l):
                last = l == 2
                cold_gathers(l)
                scatter_steps(l, n_hot[l], n_hot[l])
                for c in range(NCH):
                    cold_matmuls_chunk(l, c)
                    d_el = DL[l]
                    t = tpool.tile([T, CCHUNK], f32, tag="fmpost")
                    if not last:
                        drow = disrow_sb if apply_b2 else disrow2_sb
                        nc.vector.tensor_tensor(
                            t[:d_el, :], fm[c][:d_el, :],
                            drow[:d_el, c * CCHUNK:(c + 1) * CCHUNK], mult)
                        nc.scalar.activation(
                            h2fm[:, c * CCHUNK:(c + 1) * CCHUNK],
                            t[:d_el, :], relu, bias=b2_sb[:, :])
                        for j in range(4 * c, 4 * c + 4):
                            zp3 = ps_z.tile([T, 2 * DL[0]], f32, tag="zp")
                            nc.tensor.matmul(
                                zp3[:, :DL[2]], h2fm[:, j * T:(j + 1) * T],
                                w3_sb[:, :], start=True, stop=True)
                            zb3 = zbpool.tile([T, DPAD[2]], bf16, tag="zb3")
                            if apply_b2:
                                nc.vector.tensor_scalar(
                                    zb3[:, :DL[2]], zp3[:, :DL[2]],
                                    dis_sb[:, j:j + 1], None, mult)
                            else:
                                nc.scalar.activation(
                                    zb3[:, :DL[2]], zp3[:, :DL[2]], fcopy)
                            z_store(2, j, zb3)
                    else:
                        nc.vector.tensor_tensor(
                            t[:d_el, :], fm[c][:d_el, :],
                            disrow_sb[:d_el, c * CCHUNK:(c + 1) * CCHUNK],
                            mult)
                        if apply_b3:
                            nc.scalar.activation(
                                outfm[:DL[2], c * CCHUNK:(c + 1) * CCHUNK],
                                t[:DL[2], :], fcopy, bias=b3_sb[:DL[2], :])
                        else:
                            nc.scalar.activation(
                                outfm[:DL[2], c * CCHUNK:(c + 1) * CCHUNK],
                                t[:DL[2], :], fcopy)
                        for j in range(4 * c, 4 * c + 4):
                            tpf = ps_z.tile([T, 2 * DL[0]], f32, tag="zp")
                            nc.tensor.matmul(
                                tpf[:, :DL[2]],
                                outfm[:DL[2], j * T:(j + 1) * T],
                                identf_sb[:DL[2], :DL[2]],
                                is_transpose=True)
                            ot = htpool.tile([T, DL[2]], f32, tag="ot")
                            nc.scalar.activation(ot[:, :], tpf[:, :DL[2]],
                                                 fcopy)
                            nc.sync.dma_start(out=out_v[:, j, :],
                                              in_=ot[:, :])

            layer_tail(1)
            scat["pos"] = 0
            layer_tail(2)

    nc.compile()
    return nc


# ----------------------------------------------------------------------------
# Host-side preprocessing
# ----------------------------------------------------------------------------
def _band_node_order(outdeg, indeg):
    by_out = np.argsort(-outdeg, kind="stable")
    node_order = -np.ones(NTAB, np.int64)
    new_pos = np.zeros(N_NODES, np.int64)
    band_sz = N_CORES * T
    for k in range(NT):
        band = by_out[k * band_sz:(k + 1) * band_sz]
        band = band[np.argsort(-indeg[band], kind="stable")]
        fill = np.zeros(N_CORES, np.int64)
        b = 0
        direction = 1
        for node in band:
            pos = b * SHARD + k * T + fill[b]
            node_order[pos] = node
            new_pos[node] = pos
            fill[b] += 1
            b += direction
            if b == N_CORES:
                b = N_CORES - 1
                direction = -1
            elif b < 0:
                b = 0
                direction = 1
    return node_order, new_pos


def _group_pack(core_s, grp, ngrp, row_s, slot_s, CNT_flat, ioff_flat,
                boff_flat, mult_s=None):
    grp_start = np.zeros(N_CORES * ngrp + 1, np.int64)
    np.add.at(grp_start, core_s * ngrp + grp + 1, 1)
    grp_start = np.cumsum(grp_start)
    rank = np.arange(len(grp)) - grp_start[core_s * ngrp + grp]
    cnt_np = np.array(CNT_flat)
    ioff_np = np.array(ioff_flat)
    boff_np = np.array(boff_flat)
    epos = ioff_np[grp] * 16 + rank
    blk = boff_np[grp] + rank // T
    lane = rank % T
    idxcols = int(ioff_np[-1] + cnt_np[-1] // 16)
    totblk = int(boff_np[-1] + (cnt_np[-1] + T - 1) // T)
    idx_cores, sel_cores = [], []
    for c in range(N_CORES):
        m = core_s == c
        flat = np.zeros(idxcols * 16, np.int16)
        flat[epos[m]] = row_s[m].astype(np.int16)
        wrapped = np.tile(flat.reshape(idxcols, 16).T, (8, 1))
        idx_cores.append(np.ascontiguousarray(wrapped.astype(np.int16)))
        selc = np.zeros((totblk, T, T), np.uint8)
        selc[blk[m], lane[m], slot_s[m]] = (
            1 if mult_s is None else mult_s[m])
        sel_cores.append(np.ascontiguousarray(
            selc.transpose(1, 0, 2).reshape(T, totblk * T)).astype(FP8))
    return idx_cores, sel_cores


def _preprocess(edge_index):
    src = np.asarray(edge_index[0], dtype=np.int64)
    dst = np.asarray(edge_index[1], dtype=np.int64)
    indeg = np.bincount(dst, minlength=N_NODES).astype(np.float64) + 1.0
    outdeg = np.bincount(src, minlength=N_NODES).astype(np.float64)
    dis_full = 1.0 / np.sqrt(indeg)

    node_order, new_pos = _band_node_order(outdeg, indeg)

    spos = new_pos[src]
    dpos = new_pos[dst]
    core = dpos // SHARD
    tile = (dpos % SHARD) // T
    slot = dpos % T

    # ---- L1: one gather group per (core, dst tile); srow = global row ----
    key = ((core * NT + tile) * 32768 + spos) * T + slot
    uniq, uidx, ucnt = np.unique(key, return_index=True, return_counts=True)
    core_u = core[uidx]
    tile_u = tile[uidx]
    slot_u = slot[uidx]
    srow_u = spos[uidx]
    counts = np.zeros((N_CORES, NT), np.int64)
    np.add.at(counts, (core_u, tile_u), 1)
    CNT = [max(16, _ru16(counts[:, j].max())) for j in range(NT)]
    _, boff_f, ioff_f, _, _ = _offsets([CNT])

    order = np.lexsort((slot_u, tile_u, core_u))
    idx_cores, sel_cores = _group_pack(
        core_u[order], tile_u[order], NT, srow_u[order], slot_u[order],
        CNT, ioff_f, boff_f, mult_s=ucnt[order])

    # ---- per-layer cold edges (src tile >= JC[l] -> AG chunk 2) ----
    loop_pos = new_pos[node_order[node_order >= 0]]
    s_all = np.concatenate([spos, loop_pos])
    d_all = np.concatenate([dpos, loop_pos])
    stile_all = (s_all % SHARD) // T
    idxc_cores, selc_cores, CNTC = {}, {}, {}
    for l in (1, 2):
        jc = JC[l]
        cold_m = stile_all >= jc
        sc = s_all[cold_m]
        dc = d_all[cold_m]
        ccore = dc // SHARD
        ctile = (dc % SHARD) // T
        cslot = dc % T
        crow = (sc // SHARD) * ((NT - jc) * T) + (sc % SHARD) - jc * T
        key2 = (((ccore * NT + ctile) * 32768 + crow) * T + cslot)
        uq2, ui2, uc2 = np.unique(key2, return_index=True,
                                  return_counts=True)
        ccore_u = ccore[ui2]
        ctile_u = ctile[ui2]
        cslot_u = cslot[ui2]
        crow_u = crow[ui2]
        counts2 = np.zeros((N_CORES, NT), np.int64)
        np.add.at(counts2, (ccore_u, ctile_u), 1)
        CNTC[l] = [max(16, _ru16(counts2[:, j].max())) for j in range(NT)]
        _, boff2_f, ioff2_f, _, _ = _offsets([CNTC[l]])
        order2 = np.lexsort((cslot_u, ctile_u, ccore_u))
        idxc_cores[l], selc_cores[l] = _group_pack(
            ccore_u[order2], ctile_u[order2], NT, crow_u[order2],
            cslot_u[order2], CNTC[l], ioff2_f, boff2_f, mult_s=uc2[order2])

    # ---- S matrices: hot srcs only, rows in [a | b1-hot] order ----
    HALF_A = HT_A * T
    hot_m = stile_all < JCUT
    sh = s_all[hot_m]
    dh = d_all[hot_m]
    sh_tile = (sh % SHARD) // T
    sh_core = sh // SHARD
    sh_off = sh % T
    in_a = sh_tile < HT_A
    srow_glob = np.where(
        in_a,
        sh_core * HALF_A + sh_tile * T + sh_off,
        N_CORES * HALF_A + sh_core * (HOTB * T)
        + (sh_tile - HT_A) * T + sh_off)
    dcore_h = dh // SHARD
    dloc_h = dh % SHARD
    smat_cores = []
    for c in range(N_CORES):
        m = dcore_h == c
        S = np.zeros((NTAB, SHARD), np.uint8)
        np.add.at(S, (srow_glob[m], dloc_h[m]), 1)
        smat_cores.append(S.astype(FP8))

    dis_cores, disrow_cores = [], []
    for c in range(N_CORES):
        slots = node_order[c * SHARD:(c + 1) * SHARD]
        dis_c = np.where(slots >= 0, dis_full[np.maximum(slots, 0)], 0.0)
        dis_cores.append(np.ascontiguousarray(
            dis_c.reshape(NT, T).T).astype(np.float32))
        disrow_cores.append(np.ascontiguousarray(
            np.tile(dis_c[None, :], (T, 1))).astype(np.float32))

    return (idx_cores, sel_cores, idxc_cores, selc_cores, dis_cores,
            disrow_cores, smat_cores, CNT, CNTC, node_order, dis_full)


def _make_in_maps(x, W1, b1, W2, b2, W3, b3, edge_index):
    (idx_cores, sel_cores, idxc_cores, selc_cores, dis_cores, disrow_cores,
     smat_cores, CNT, CNTC, node_order, dis_full) = _preprocess(edge_index)

    x = np.asarray(x, np.float32)
    w1b = np.asarray(W1, np.float32).astype(BF16)
    w2b = np.asarray(W2, np.float32).astype(BF16)
    w3b = np.asarray(W3, np.float32).astype(BF16)
    b1f = np.asarray(b1, np.float32)
    b2f = np.asarray(b2, np.float32)
    b3f = np.asarray(b3, np.float32)
    b1c2 = np.zeros((T, 2), np.float32)
    b1c2[:, 0] = b1f[:T]
    b1c2[:, 1] = b1f[T:]
    b2col = np.zeros((T, 1), np.float32)
    b2col[:DL[1], 0] = b2f
    b3col = np.zeros((T, 1), np.float32)
    b3col[:DL[2], 0] = b3f
    identb = np.eye(T, dtype=BF16)
    identf = np.eye(T, dtype=np.float32)
    apply_b1 = bool(np.any(b1f))
    apply_b2 = bool(np.any(b2f))
    apply_b3 = bool(np.any(b3f))

    # global dis-prescaled X table (identical on every core)
    valid = node_order >= 0
    xtab = np.zeros((NTAB, D0), np.float32)
    xtab[valid] = (x[node_order[valid]]
                   * dis_full[node_order[valid]][:, None])
    xtab = xtab.astype(BF16)

    in_maps = []
    for c in range(N_CORES):
        xs = xtab[c * SHARD:(c + 1) * SHARD].reshape(NT, T, D0)
        xnm = np.ascontiguousarray(
            xs.transpose(1, 0, 2).reshape(T, NT * D0))
        in_maps.append({
            "xtab": xtab, "xnm": xnm,
            "w1": w1b, "w2": w2b, "w3": w3b,
            "b1c2": b1c2, "b2col": b2col, "b3col": b3col,
            "dis": dis_cores[c],
            "disrow": disrow_cores[c].astype(BF16),
            "disrow2": (disrow_cores[c] ** 2).astype(BF16),
            "identb": identb, "identf": identf,
            "idx": idx_cores[c], "sel": sel_cores[c],
            "idxc1": idxc_cores[1][c], "selc1": selc_cores[1][c],
            "idxc2": idxc_cores[2][c], "selc2": selc_cores[2][c],
            "smat": smat_cores[c],
        })
    return in_maps, CNT, CNTC, node_order, apply_b1, apply_b2, apply_b3


_NC_CACHE = {}


def kernel_with_results(x, W1, b1, W2, b2, W3, b3, edge_index, trace=False):
    (in_maps, CNT, CNTC, node_order, apply_b1, apply_b2,
     apply_b3) = _make_in_maps(x, W1, b1, W2, b2, W3, b3, edge_index)
    key = (tuple(CNT), tuple(CNTC[1]), tuple(CNTC[2]),
           apply_b1, apply_b2, apply_b3)
    if key not in _NC_CACHE:
        _NC_CACHE[key] = _build_nc(CNT, CNTC, apply_b1, apply_b2, apply_b3)
    nc = _NC_CACHE[key]
    res = run_bass_kernel_spmd(
        nc, in_maps, core_ids=list(range(N_CORES)), trace=trace)
    rows = np.concatenate(
        [np.asarray(res.results[c]["out"]) for c in range(N_CORES)], axis=0)
    full = np.zeros((N_NODES, rows.shape[1]), np.float32)
    real = node_order >= 0
    full[node_order[real]] = rows[real]
    return full, res


def kernel(x, W1, b1, W2, b2, W3, b3, edge_index):
    full, _ = kernel_with_results(x, W1, b1, W2, b2, W3, b3, edge_index)
    return full


# revision 22
# speedup vs baseline: 1.0601x; 1.0601x over previous
"""GCN (3-layer, PyG GCNConv-style) forward pass on 8 Trainium2 NeuronCores.

Architecture v6 (L1 direct from replicated X-table; no AG-L1):
  - Aggregation commutes with the GEMM: segsum(X~[src]) @ W1 ==
    segsum((X~ W1)[src]) where X~ = dis*x.  Every core receives the full
    x, so each core holds the complete dis-prescaled X table from t=0 and
    L1 edge-gathers start immediately -- layer 1 needs NO AllGather and
    no Z1 pre-GEMM phase.
  - L1 per dst tile (feature-major): SWDGE dma_gather of unique (src,slot)
    rows from the X table + fp8-selector matmuls accumulate
    A1fm [256 x 128dst] in PSUM (self-loop via identity matmul on a
    node-major x tile); then W1 GEMM, dis/bias/relu post, W2 GEMM and a
    single PE transpose produce node-major zb2 for the L2 table.
  - Each layer AllGathers its Z table in two chunks -- hot (tiles <
    JCUT, feeds the PE-scatter) and cold -- fired via per-chunk counters;
    L1 processes hot dst tiles first so the hot AG dispatches ~halfway
    through L1 and the 88-step scatter fully overlaps the cold gathers.
  - Layers 2/3 keep the hot/cold split: hot source tiles via PE-scatter
    (Z stationary, 0/1 fp8 adjacency slices streamed as moving operand
    into a feature-major PSUM [d x 2560] in five 512-col banks), cold
    source tiles via SWDGE gather + per-dst-tile selector matmuls.
  - The layer tail is pipelined per 512-col chunk: close chunk c (cold
    matmuls) -> feature-major post -> next-layer GEMM -> AllGather chunk
    fires as soon as its tiles are stored.  Final output is PE-transposed
    back to node-major.
"""

import sys

import numpy as np

sys.path.insert(0, "/opt/trn_rl_repo")

import ml_dtypes  # noqa: E402

import concourse.bass as bass  # noqa: E402
import concourse.bacc as bacc  # noqa: E402
import concourse.mybir as mybir  # noqa: E402
from concourse.bass_utils import run_bass_kernel_spmd  # noqa: E402
from concourse.library_config import mlp as _mlp_lib  # noqa: E402
from concourse.tile import TileContext  # noqa: E402
from concourse.tile_rust import add_dep_helper  # noqa: E402

BF16 = ml_dtypes.bfloat16
FP8 = ml_dtypes.float8_e4m3

# ----------------------------------------------------------------------------
# Problem configuration (hardcoded for nn_Encoder_17386027614431)
# ----------------------------------------------------------------------------
N_NODES = 20000
N_CORES = 8
T = 128
NT = 20                  # dst tiles per core
SHARD = NT * T           # 2560
NTAB = N_CORES * SHARD   # 20480 table rows
D0 = 256
DL = [256, 128, 64]      # per-layer output dims
HT_A = 11                # tiles in AllGather chunk a (= all hot tiles)
JCUT = 11                # L2: tiles >= JCUT are cold (edges via gather)
JCUT3 = 11               # L3: tiles >= JCUT3 are cold
JC = {1: JCUT, 2: JCUT3}
SA_TILES = N_CORES * HT_A          # 32 src tiles in chunk-a table
HOTB = JCUT - HT_A                 # hot b1 tiles per core in smat layout
B1N = {1: JCUT - HT_A, 2: JCUT3 - HT_A}   # b1 tiles per core per layer
DPAD = [256, 128, 128]             # table row widths (L3 padded)
CCHUNK = 512                       # psum bank columns (f32)
NCH = SHARD // CCHUNK              # 5 feature-major column chunks
# AG chunk tile ranges per layer (index 0 unused in v6): one hot chunk
# (feeds the PE-scatter) + one cold chunk (feeds the cold gathers).
AGCH = [
    None,
    [(0, JCUT), (JCUT, NT)],
    [(0, JCUT3), (JCUT3, NT)],
]
# L1 dst-tile processing order: natural -- hot tiles (0..JCUT) complete
# first so the hot AG dispatches ~halfway through L1, then the cold tiles.
TILE_ORDER = list(range(NT))


def _ru16(x):
    return (int(x) + 15) // 16 * 16


def _offsets(cnt2d):
    flat = [c for row in cnt2d for c in row]
    nbl = [(c + T - 1) // T for c in flat]
    boff, ioff = [], []
    ob = oi = 0
    for c, nb in zip(flat, nbl):
        boff.append(ob)
        ioff.append(oi)
        ob += nb
        oi += c // 16
    return nbl, boff, ioff, ob, oi


def _build_nc(CNT, CNTC, apply_b1, apply_b2, apply_b3):
    f32 = mybir.dt.float32
    bf16 = mybir.dt.bfloat16
    fp8 = mybir.dt.float8e4
    i16 = mybir.dt.int16
    mult = mybir.AluOpType.mult
    add = mybir.AluOpType.add
    relu = mybir.ActivationFunctionType.Relu
    fcopy = mybir.ActivationFunctionType.Copy

    nbl, boff, ioff, totblk, idxcols = _offsets([CNT])
    nblc, boffc, ioffc, totblkc, idxcolsc = {}, {}, {}, {}, {}
    for l in (1, 2):
        nblc[l], boffc[l], ioffc[l], totblkc[l], idxcolsc[l] = _offsets(
            [CNTC[l]])
    maxnb = max(nbl)
    maxnbc = max(max(nblc[1]), max(nblc[2]))

    nc = bacc.Bacc("TRN2", num_devices=N_CORES, num_swdge_queues=4)

    # ---- kernel I/O ----
    xtab = nc.dram_tensor("xtab", [NTAB, D0], bf16, kind="ExternalInput")
    xnm = nc.dram_tensor("xnm", [T, NT * D0], bf16, kind="ExternalInput")
    w1 = nc.dram_tensor("w1", [D0, DL[0]], bf16, kind="ExternalInput")
    w2 = nc.dram_tensor("w2", [DL[0], DL[1]], bf16, kind="ExternalInput")
    w3 = nc.dram_tensor("w3", [DL[1], DL[2]], bf16, kind="ExternalInput")
    b1c2 = nc.dram_tensor("b1c2", [T, 2], f32, kind="ExternalInput")
    b2col = nc.dram_tensor("b2col", [T, 1], f32, kind="ExternalInput")
    b3col = nc.dram_tensor("b3col", [T, 1], f32, kind="ExternalInput")
    dis = nc.dram_tensor("dis", [T, NT], f32, kind="ExternalInput")
    disrow = nc.dram_tensor("disrow", [T, SHARD], bf16,
                            kind="ExternalInput")
    disrow2 = nc.dram_tensor("disrow2", [T, SHARD], bf16,
                             kind="ExternalInput")
    identb = nc.dram_tensor("identb", [T, T], bf16, kind="ExternalInput")
    identf = nc.dram_tensor("identf", [T, T], f32, kind="ExternalInput")
    idx = nc.dram_tensor("idx", [T, idxcols], i16, kind="ExternalInput")
    sel = nc.dram_tensor("sel", [T, totblk * T], fp8, kind="ExternalInput")
    idxc = {l: nc.dram_tensor(f"idxc{l}", [T, idxcolsc[l]], i16,
                              kind="ExternalInput") for l in (1, 2)}
    selc = {l: nc.dram_tensor(f"selc{l}", [T, totblkc[l] * T], fp8,
                              kind="ExternalInput") for l in (1, 2)}
    smat = nc.dram_tensor("smat", [NTAB, SHARD], fp8, kind="ExternalInput")
    out = nc.dram_tensor("out", [SHARD, DL[2]], f32, kind="ExternalOutput")

    # ---- internal DRAM for collectives (per layer, per AG chunk) ----
    agin = {l: [] for l in (1, 2)}
    agout = {l: [] for l in (1, 2)}
    for l in (1, 2):
        for k, (j0, j1) in enumerate(AGCH[l]):
            rows = (j1 - j0) * T
            agin[l].append(nc.dram_tensor(
                f"agin{l}_{k}", [rows, DPAD[l]], bf16))
            agout[l].append(nc.dram_tensor(
                f"agout{l}_{k}", [N_CORES * rows, DPAD[l]], bf16,
                addr_space="Shared"))
    rg = [list(range(N_CORES))]

    with TileContext(nc) as tc:
        nc.gpsimd.load_library(_mlp_lib)

        with (
            tc.tile_pool(name="const", bufs=1) as cpool,
            tc.tile_pool(name="sb", bufs=10) as sbpool,        # S stream
            tc.tile_pool(name="zsb", bufs=3) as zspool,       # Z stationary
            tc.tile_pool(name="selp", bufs=3) as selpool,
            tc.tile_pool(name="hp", bufs=2) as hpool,
            tc.tile_pool(name="htp", bufs=3) as htpool,
            tc.tile_pool(name="tmp", bufs=3) as tpool,
            tc.tile_pool(name="zbp", bufs=3) as zbpool,
            tc.tile_pool(name="ps_z", bufs=1, space="PSUM") as ps_z,
            tc.tile_pool(name="ps_agg", bufs=1, space="PSUM") as ps_agg,
            tc.tile_pool(name="ps_t", bufs=1, space="PSUM") as ps_t,
            tc.tile_pool(name="ps_fm", bufs=1, space="PSUM") as ps_fm,
        ):
            # ---- constants (idx/sel path first so gathers start at t=0) ----
            def load_const(dram_h, shape, dtype):
                t = cpool.tile(shape, dtype, tag=f"c_{dram_h.name}")
                nc.sync.dma_start(out=t[:, :], in_=dram_h.ap())
                return t

            def load_const_chunked(dram_h, inner, dtype):
                cs = dram_h.shape[0] // T
                t = cpool.tile([T, cs * inner], dtype, tag=f"c_{dram_h.name}")
                nc.sync.dma_start(
                    out=t.rearrange("p (c n) -> p c n", c=cs),
                    in_=dram_h.ap().rearrange("(c p) n -> p c n", p=T),
                )
                return t

            idx_sb = load_const(idx, [T, idxcols], i16)
            xnm_sb = load_const(xnm, [T, NT * D0], bf16)
            w1_sb = load_const_chunked(w1, DL[0], bf16)
            identb_sb = load_const(identb, [T, T], bf16)
            disrow_sb = load_const(disrow, [T, SHARD], bf16)
            disrow2_sb = load_const(disrow2, [T, SHARD], bf16)
            b1c2_sb = load_const(b1c2, [T, 2], f32)
            idxc_sb = {l: load_const(idxc[l], [T, idxcolsc[l]], i16)
                       for l in (1, 2)}
            w2_sb = load_const_chunked(w2, DL[1], bf16)
            w3_sb = load_const(w3, [DL[1], DL[2]], bf16)
            dis_sb = load_const(dis, [T, NT], f32)
            b2_sb = load_const(b2col, [T, 1], f32)
            b3_sb = load_const(b3col, [T, 1], f32)
            identf_sb = load_const(identf, [T, T], f32)

            # persistent buffers
            gbuf = [cpool.tile([T, maxnb * D0], bf16, tag=f"g{i}",
                               name=f"gbuf{i}") for i in range(4)]
            for g in gbuf:
                nc.vector.memset(g[:, :], 0.0)
            h2fm = cpool.tile([T, SHARD], bf16, tag="h2fm")
            outfm = cpool.tile([T, SHARD], f32, tag="outfm")
            fm = [ps_fm.tile([T, CCHUNK], f32, tag=f"fm{c}",
                             name=f"fm{c}") for c in range(NCH)]
            cbuf = [cpool.tile([T, maxnbc * DPAD[1]], bf16, tag=f"cb{j}",
                               name=f"cbuf{j}") for j in range(NT)]
            for cb in cbuf:
                nc.vector.memset(cb[:, :], 0.0)

            agin_v = {l: [agin[l][k].ap().rearrange("(n p) d -> p n d", p=T)
                          for k in range(len(AGCH[l]))] for l in (1, 2)}
            agout_v = {l: [agout[l][k].ap().rearrange("(n p) d -> p n d", p=T)
                           for k in range(len(AGCH[l]))] for l in (1, 2)}
            smat_v = smat.ap().rearrange("(s p) d -> p s d", p=T)
            out_v = out.ap().rearrange("(n p) d -> p n d", p=T)

            ag_insts = {l: [None] * len(AGCH[l]) for l in (1, 2)}
            agin_dmas = {l: [[] for _ in AGCH[l]] for l in (1, 2)}
            ag_left = {l: [j1 - j0 for (j0, j1) in AGCH[l]] for l in (1, 2)}

            def issue_ag(l, k):
                cc = nc.gpsimd.collective_compute(
                    "AllGather",
                    mybir.AluOpType.bypass,
                    replica_groups=rg,
                    ins=[agin[l][k].ap().opt()],
                    outs=[agout[l][k].ap().opt()],
                )
                for d in agin_dmas[l][k]:
                    add_dep_helper(cc.ins, d.ins, reason=f"ag{l}.{k}")
                ag_insts[l][k] = cc

            def z_store(l, j, zb):
                for k, (j0, j1) in enumerate(AGCH[l]):
                    if j0 <= j < j1:
                        break
                d = nc.sync.dma_start(
                    out=agin_v[l][k][:, j - j0, :], in_=zb[:, :])
                agin_dmas[l][k].append(d)
                ag_left[l][k] -= 1
                if ag_left[l][k] == 0:
                    issue_ag(l, k)

            # ---- cold gathers / matmuls for L2/L3 ----
            gq = [0]

            def cold_gathers(l):
                for j in range(NT):
                    cnt = CNTC[l][j]
                    gt3 = cbuf[j][:, :nblc[l][j] * DPAD[l]].rearrange(
                        "p (n d) -> p n d", d=DPAD[l])
                    g = nc.gpsimd.dma_gather(
                        gt3,
                        agout[l][1].ap(),
                        idxc_sb[l][:, ioffc[l][j]:ioffc[l][j] + cnt // 16],
                        cnt, cnt, DPAD[l],
                        single_packet=False,
                        queue_num=gq[0] % 4,
                    )
                    gq[0] += 1
                    add_dep_helper(g.ins, ag_insts[l][1].ins, reason="cg ag")

            def cold_matmuls_chunk(l, c):
                """Fold cold edges of dst tiles 4c..4c+3 into fm[c]; the last
                one closes the accumulation group."""
                d_el = DL[l]
                for j in range(4 * c, 4 * c + 4):
                    nb = nblc[l][j]
                    gt3 = cbuf[j][:, :nb * DPAD[l]].rearrange(
                        "p (n d) -> p n d", d=DPAD[l])
                    st = selpool.tile([T, max(maxnb, maxnbc) * T], fp8,
                                      tag="sel")
                    nc.sync.dma_start(
                        out=st[:, :nb * T],
                        in_=selc[l][:, boffc[l][j] * T:
                                   (boffc[l][j] + nb) * T])
                    r = (j % 4) * T
                    for b in range(nb):
                        nc.tensor.matmul(
                            fm[c][:d_el, r:r + T],
                            gt3[:, b, :d_el],
                            st[:, b * T:(b + 1) * T],
                            start=False,
                            stop=(j % 4 == 3 and b == nb - 1),
                            skip_group_check=True)

            # ---- hot scatter: chunk-a tiles then per-core b1 stripes; S
            # fetched two tiles per DMA on the Activation HWDGE queue. ----
            fetch_plan = {}           # per layer: (smat_row0, ntiles)
            hot_steps = {}
            for l in (1, 2):
                fp_ = []
                for g2 in range(SA_TILES // 2):
                    fp_.append((2 * g2, 2))
                for core in range(N_CORES):
                    base = SA_TILES + core * HOTB
                    k = 0
                    while k < B1N[l]:
                        n = min(2, B1N[l] - k)
                        fp_.append((base + k, n))
                        k += n
                fetch_plan[l] = fp_
                hs = []
                for fi, (r0, n) in enumerate(fp_):
                    for k in range(n):
                        hs.append((fi, k))
                hot_steps[l] = hs
            n_hot = {l: len(hot_steps[l]) for l in (1, 2)}

            scat = {"pos": 0, "zsb": None, "stile": None}

            def scatter_steps(l, n, limit):
                d_el = DL[l]
                dp = DPAD[l]
                b1n = B1N[l]
                while n > 0 and scat["pos"] < limit:
                    pos = scat["pos"]
                    fi, k = hot_steps[l][pos]
                    r0, fn = fetch_plan[l][fi]
                    if pos < SA_TILES:
                        if pos % 8 == 0:
                            zsb = zspool.tile([T, 8 * dp], bf16,
                                              tag=f"zsa{l}")
                            d = nc.sync.dma_start(
                                out=zsb.rearrange("p (n d) -> p n d", d=dp),
                                in_=agout_v[l][0][:, pos:pos + 8, :])
                            add_dep_helper(d.ins, ag_insts[l][0].ins,
                                           reason="zs ag")
                            scat["zsb"] = zsb
                        zk = pos % 8
                    else:
                        p = pos - SA_TILES
                        if p % b1n == 0:
                            core = p // b1n
                            zsb = zspool.tile([T, b1n * dp], bf16,
                                              tag=f"zsb{l}")
                            d = nc.sync.dma_start(
                                out=zsb.rearrange("p (n d) -> p n d", d=dp),
                                in_=agout_v[l][1][:, core * b1n:
                                                  (core + 1) * b1n, :])
                            add_dep_helper(d.ins, ag_insts[l][1].ins,
                                           reason="zs ag")
                            scat["zsb"] = zsb
                        zk = p % b1n
                    if k == 0:
                        stile = sbpool.tile([T, 2 * SHARD], fp8, tag="sm")
                        nc.scalar.dma_start(
                            out=stile[:, :fn * SHARD].rearrange(
                                "p (n d) -> p n d", d=SHARD),
                            in_=smat_v[:, r0:r0 + fn, :])
                        scat["stile"] = stile
                    stile = scat["stile"]
                    zsb = scat["zsb"]
                    for c in range(NCH):
                        nc.tensor.matmul(
                            fm[c][:d_el, :],
                            zsb[:, zk * dp:zk * dp + d_el],
                            stile[:, k * SHARD + c * CCHUNK:
                                  k * SHARD + (c + 1) * CCHUNK],
                            start=(pos == 0), stop=False,
                            skip_group_check=True)
                    scat["pos"] = pos + 1
                    n -= 1

            # ================= Layer 1 (from the replicated X table) =======
            for ji, j in enumerate(TILE_ORDER):
                cnt = CNT[j]
                nb = nbl[j]
                gslot = ji % 4
                gt3 = gbuf[gslot][:, :nb * D0].rearrange(
                    "p (n d) -> p n d", d=D0)
                nc.gpsimd.dma_gather(
                    gt3,
                    xtab.ap(),
                    idx_sb[:, ioff[j]:ioff[j] + cnt // 16],
                    cnt, cnt, D0,
                    single_packet=False,
                    queue_num=gq[0] % 4,
                )
                gq[0] += 1
                st = selpool.tile([T, max(maxnb, maxnbc) * T], fp8, tag="sel")
                nc.sync.dma_start(
                    out=st[:, :nb * T],
                    in_=sel[:, boff[j] * T:(boff[j] + nb) * T])
                # A1fm [256feat x 128dst] in two halves; self-loop first
                a1 = ps_agg.tile([T, 2 * T], f32, tag="agg")
                for h in range(2):
                    nc.tensor.matmul(
                        a1[:, h * T:(h + 1) * T],
                        xnm_sb[:, j * D0 + h * T:j * D0 + (h + 1) * T],
                        identb_sb[:, :],
                        start=True, stop=False)
                    for b in range(nb):
                        nc.tensor.matmul(
                            a1[:, h * T:(h + 1) * T],
                            gt3[:, b, h * T:(h + 1) * T],
                            st[:, b * T:(b + 1) * T],
                            start=False, stop=(b == nb - 1))
                a1sb = tpool.tile([T, 2 * T], bf16, tag="a1")
                nc.scalar.activation(a1sb[:, :], a1[:, :], fcopy)
                # h1fm = relu(dis^2 * (A1 @ W1)) (or dis/b1 variant)
                zp = ps_z.tile([T, 2 * DL[0]], f32, tag="zp")
                for h in range(2):
                    for cch in range(2):
                        nc.tensor.matmul(
                            zp[:, h * T:(h + 1) * T],
                            w1_sb[:, cch * D0 + h * T:cch * D0 + (h + 1) * T],
                            a1sb[:, cch * T:(cch + 1) * T],
                            start=(cch == 0), stop=(cch == 1))
                t1 = tpool.tile([T, 2 * T], f32, tag="post")
                drow = disrow_sb if apply_b1 else disrow2_sb
                for h in range(2):
                    nc.vector.tensor_tensor(
                        t1[:, h * T:(h + 1) * T], zp[:, h * T:(h + 1) * T],
                        drow[:, j * T:(j + 1) * T], mult)
                h1sb = hpool.tile([T, 2 * T], bf16, tag="h1")
                if apply_b1:
                    for h in range(2):
                        nc.scalar.activation(
                            h1sb[:, h * T:(h + 1) * T],
                            t1[:, h * T:(h + 1) * T], relu,
                            bias=b1c2_sb[:, h:h + 1])
                else:
                    nc.scalar.activation(h1sb[:, :], t1[:, :], relu)
                # zb2fm = (h1 @ W2)^T then one transpose to node-major
                for cch in range(2):
                    nc.tensor.matmul(
                        zp[:, 2 * T:2 * T + DL[1]],
                        w2_sb[:, cch * DL[1]:(cch + 1) * DL[1]],
                        h1sb[:, cch * T:(cch + 1) * T],
                        start=(cch == 0), stop=(cch == 1))
                zfs = zbpool.tile([T, DL[1]], bf16, tag="z2f")
                if apply_b1:
                    nc.vector.tensor_tensor(
                        zfs[:, :], zp[:, 2 * T:2 * T + DL[1]],
                        disrow_sb[:, j * T:(j + 1) * T], mult)
                else:
                    nc.scalar.activation(
                        zfs[:, :], zp[:, 2 * T:2 * T + DL[1]], fcopy)
                tp = ps_t.tile([T, T], bf16, tag="tp")
                nc.tensor.matmul(tp[:, :], zfs[:, :], identb_sb[:, :],
                                 is_transpose=True)
                zb2 = zbpool.tile([T, DL[1]], bf16, tag="zb2")
                nc.scalar.activation(zb2[:, :], tp[:, :], fcopy)
                z_store(1, j, zb2)

            # ---- layer tails: finish aggregation, pipeline per chunk ----
            def layer_tail(l):
                last = l == 2
                cold_gathers(l)
                scatter_steps(l, n_hot[l], n_hot[l])
                for c in range(NCH):
                    cold_matmuls_chunk(l, c)
                    d_el = DL[l]
                    t = tpool.tile([T, CCHUNK], f32, tag="fmpost")
                    if not last:
                        drow = disrow_sb if apply_b2 else disrow2_sb
                        nc.vector.tensor_tensor(
                            t[:d_el, :], fm[c][:d_el, :],
                            drow[:d_el, c * CCHUNK:(c + 1) * CCHUNK], mult)
                        nc.scalar.activation(
                            h2fm[:, c * CCHUNK:(c + 1) * CCHUNK],
                            t[:d_el, :], relu, bias=b2_sb[:, :])
                        for j in range(4 * c, 4 * c + 4):
                            zp3 = ps_z.tile([T, 2 * DL[0]], f32, tag="zp")
                            nc.tensor.matmul(
                                zp3[:, :DL[2]], h2fm[:, j * T:(j + 1) * T],
                                w3_sb[:, :], start=True, stop=True)
                            zb3 = zbpool.tile([T, DPAD[2]], bf16, tag="zb3")
                            if apply_b2:
                                nc.vector.tensor_scalar(
                                    zb3[:, :DL[2]], zp3[:, :DL[2]],
                                    dis_sb[:, j:j + 1], None, mult)
                            else:
                                nc.scalar.activation(
                                    zb3[:, :DL[2]], zp3[:, :DL[2]], fcopy)
                            z_store(2, j, zb3)
                    else:
                        nc.vector.tensor_tensor(
                            t[:d_el, :], fm[c][:d_el, :],
                            disrow_sb[:d_el, c * CCHUNK:(c + 1) * CCHUNK],
                            mult)
                        if apply_b3:
                            nc.scalar.activation(
                                outfm[:DL[2], c * CCHUNK:(c + 1) * CCHUNK],
                                t[:DL[2], :], fcopy, bias=b3_sb[:DL[2], :])
                        else:
                            nc.scalar.activation(
                                outfm[:DL[2], c * CCHUNK:(c + 1) * CCHUNK],
                                t[:DL[2], :], fcopy)
                        for j in range(4 * c, 4 * c + 4):
                            tpf = ps_z.tile([T, 2 * DL[0]], f32, tag="zp")
                            nc.tensor.matmul(
                                tpf[:, :DL[2]],
                                outfm[:DL[2], j * T:(j + 1) * T],
                                identf_sb[:DL[2], :DL[2]],
                                is_transpose=True)
                            ot = htpool.tile([T, DL[2]], f32, tag="ot")
                            nc.scalar.activation(ot[:, :], tpf[:, :DL[2]],
                                                 fcopy)
                            nc.sync.dma_start(out=out_v[:, j, :],
                                              in_=ot[:, :])

            layer_tail(1)
            scat["pos"] = 0
            layer_tail(2)

    nc.compile()
    return nc


# ----------------------------------------------------------------------------
# Host-side preprocessing
# ----------------------------------------------------------------------------
def _band_node_order(outdeg, indeg):
    by_out = np.argsort(-outdeg, kind="stable")
    node_order = -np.ones(NTAB, np.int64)
    new_pos = np.zeros(N_NODES, np.int64)
    band_sz = N_CORES * T
    for k in range(NT):
        band = by_out[k * band_sz:(k + 1) * band_sz]
        band = band[np.argsort(-indeg[band], kind="stable")]
        fill = np.zeros(N_CORES, np.int64)
        b = 0
        direction = 1
        for node in band:
            pos = b * SHARD + k * T + fill[b]
            node_order[pos] = node
            new_pos[node] = pos
            fill[b] += 1
            b += direction
            if b == N_CORES:
                b = N_CORES - 1
                direction = -1
            elif b < 0:
                b = 0
                direction = 1
    return node_order, new_pos


def _group_pack(core_s, grp, ngrp, row_s, slot_s, CNT_flat, ioff_flat,
                boff_flat, mult_s=None):
    grp_start = np.zeros(N_CORES * ngrp + 1, np.int64)
    np.add.at(grp_start, core_s * ngrp + grp + 1, 1)
    grp_start = np.cumsum(grp_start)
    rank = np.arange(len(grp)) - grp_start[core_s * ngrp + grp]
    cnt_np = np.array(CNT_flat)
    ioff_np = np.array(ioff_flat)
    boff_np = np.array(boff_flat)
    epos = ioff_np[grp] * 16 + rank
    blk = boff_np[grp] + rank // T
    lane = rank % T
    idxcols = int(ioff_np[-1] + cnt_np[-1] // 16)
    totblk = int(boff_np[-1] + (cnt_np[-1] + T - 1) // T)
    idx_cores, sel_cores = [], []
    for c in range(N_CORES):
        m = core_s == c
        flat = np.zeros(idxcols * 16, np.int16)
        flat[epos[m]] = row_s[m].astype(np.int16)
        wrapped = np.tile(flat.reshape(idxcols, 16).T, (8, 1))
        idx_cores.append(np.ascontiguousarray(wrapped.astype(np.int16)))
        selc = np.zeros((totblk, T, T), np.uint8)
        selc[blk[m], lane[m], slot_s[m]] = (
            1 if mult_s is None else mult_s[m])
        sel_cores.append(np.ascontiguousarray(
            selc.transpose(1, 0, 2).reshape(T, totblk * T)).astype(FP8))
    return idx_cores, sel_cores


def _preprocess(edge_index):
    src = np.asarray(edge_index[0], dtype=np.int64)
    dst = np.asarray(edge_index[1], dtype=np.int64)
    indeg = np.bincount(dst, minlength=N_NODES).astype(np.float64) + 1.0
    outdeg = np.bincount(src, minlength=N_NODES).astype(np.float64)
    dis_full = 1.0 / np.sqrt(indeg)

    node_order, new_pos = _band_node_order(outdeg, indeg)

    spos = new_pos[src]
    dpos = new_pos[dst]
    core = dpos // SHARD
    tile = (dpos % SHARD) // T
    slot = dpos % T

    # ---- L1: one gather group per (core, dst tile); srow = global row ----
    key = ((core * NT + tile) * 32768 + spos) * T + slot
    uniq, uidx, ucnt = np.unique(key, return_index=True, return_counts=True)
    core_u = core[uidx]
    tile_u = tile[uidx]
    slot_u = slot[uidx]
    srow_u = spos[uidx]
    counts = np.zeros((N_CORES, NT), np.int64)
    np.add.at(counts, (core_u, tile_u), 1)
    CNT = [max(16, _ru16(counts[:, j].max())) for j in range(NT)]
    _, boff_f, ioff_f, _, _ = _offsets([CNT])

    order = np.lexsort((slot_u, tile_u, core_u))
    idx_cores, sel_cores = _group_pack(
        core_u[order], tile_u[order], NT, srow_u[order], slot_u[order],
        CNT, ioff_f, boff_f, mult_s=ucnt[order])

    # ---- per-layer cold edges (src tile >= JC[l] -> AG chunk 2) ----
    loop_pos = new_pos[node_order[node_order >= 0]]
    s_all = np.concatenate([spos, loop_pos])
    d_all = np.concatenate([dpos, loop_pos])
    stile_all = (s_all % SHARD) // T
    idxc_cores, selc_cores, CNTC = {}, {}, {}
    for l in (1, 2):
        jc = JC[l]
        cold_m = stile_all >= jc
        sc = s_all[cold_m]
        dc = d_all[cold_m]
        ccore = dc // SHARD
        ctile = (dc % SHARD) // T
        cslot = dc % T
        crow = (sc // SHARD) * ((NT - jc) * T) + (sc % SHARD) - jc * T
        key2 = (((ccore * NT + ctile) * 32768 + crow) * T + cslot)
        uq2, ui2, uc2 = np.unique(key2, return_index=True,
                                  return_counts=True)
        ccore_u = ccore[ui2]
        ctile_u = ctile[ui2]
        cslot_u = cslot[ui2]
        crow_u = crow[ui2]
        counts2 = np.zeros((N_CORES, NT), np.int64)
        np.add.at(counts2, (ccore_u, ctile_u), 1)
        CNTC[l] = [max(16, _ru16(counts2[:, j].max())) for j in range(NT)]
        _, boff2_f, ioff2_f, _, _ = _offsets([CNTC[l]])
        order2 = np.lexsort((cslot_u, ctile_u, ccore_u))
        idxc_cores[l], selc_cores[l] = _group_pack(
            ccore_u[order2], ctile_u[order2], NT, crow_u[order2],
            cslot_u[order2], CNTC[l], ioff2_f, boff2_f, mult_s=uc2[order2])

    # ---- S matrices: hot srcs only, rows in [a | b1-hot] order ----
    HALF_A = HT_A * T
    hot_m = stile_all < JCUT
    sh = s_all[hot_m]
    dh = d_all[hot_m]
    sh_tile = (sh % SHARD) // T
    sh_core = sh // SHARD
    sh_off = sh % T
    in_a = sh_tile < HT_A
    srow_glob = np.where(
        in_a,
        sh_core * HALF_A + sh_tile * T + sh_off,
        N_CORES * HALF_A + sh_core * (HOTB * T)
        + (sh_tile - HT_A) * T + sh_off)
    dcore_h = dh // SHARD
    dloc_h = dh % SHARD
    smat_cores = []
    for c in range(N_CORES):
        m = dcore_h == c
        S = np.zeros((NTAB, SHARD), np.uint8)
        np.add.at(S, (srow_glob[m], dloc_h[m]), 1)
        smat_cores.append(S.astype(FP8))

    dis_cores, disrow_cores = [], []
    for c in range(N_CORES):
        slots = node_order[c * SHARD:(c + 1) * SHARD]
        dis_c = np.where(slots >= 0, dis_full[np.maximum(slots, 0)], 0.0)
        dis_cores.append(np.ascontiguousarray(
            dis_c.reshape(NT, T).T).astype(np.float32))
        disrow_cores.append(np.ascontiguousarray(
            np.tile(dis_c[None, :], (T, 1))).astype(np.float32))

    return (idx_cores, sel_cores, idxc_cores, selc_cores, dis_cores,
            disrow_cores, smat_cores, CNT, CNTC, node_order, dis_full)


def _make_in_maps(x, W1, b1, W2, b2, W3, b3, edge_index):
    (idx_cores, sel_cores, idxc_cores, selc_cores, dis_cores, disrow_cores,
     smat_cores, CNT, CNTC, node_order, dis_full) = _preprocess(edge_index)

    x = np.asarray(x, np.float32)
    w1b = np.asarray(W1, np.float32).astype(BF16)
    w2b = np.asarray(W2, np.float32).astype(BF16)
    w3b = np.asarray(W3, np.float32).astype(BF16)
    b1f = np.asarray(b1, np.float32)
    b2f = np.asarray(b2, np.float32)
    b3f = np.asarray(b3, np.float32)
    b1c2 = np.zeros((T, 2), np.float32)
    b1c2[:, 0] = b1f[:T]
    b1c2[:, 1] = b1f[T:]
    b2col = np.zeros((T, 1), np.float32)
    b2col[:DL[1], 0] = b2f
    b3col = np.zeros((T, 1), np.float32)
    b3col[:DL[2], 0] = b3f
    identb = np.eye(T, dtype=BF16)
    identf = np.eye(T, dtype=np.float32)
    apply_b1 = bool(np.any(b1f))
    apply_b2 = bool(np.any(b2f))
    apply_b3 = bool(np.any(b3f))

    # global dis-prescaled X table (identical on every core)
    valid = node_order >= 0
    xtab = np.zeros((NTAB, D0), np.float32)
    xtab[valid] = (x[node_order[valid]]
                   * dis_full[node_order[valid]][:, None])
    xtab = xtab.astype(BF16)

    in_maps = []
    for c in range(N_CORES):
        xs = xtab[c * SHARD:(c + 1) * SHARD].reshape(NT, T, D0)
        xnm = np.ascontiguousarray(
            xs.transpose(1, 0, 2).reshape(T, NT * D0))
        in_maps.append({
            "xtab": xtab, "xnm": xnm,
            "w1": w1b, "w2": w2b, "w3": w3b,
            "b1c2": b1c2, "b2col": b2col, "b3col": b3col,
            "dis": dis_cores[c],
            "disrow": disrow_cores[c].astype(BF16),
            "disrow2": (disrow_cores[c] ** 2).astype(BF16),
            "identb": identb, "identf": identf,
            "idx": idx_cores[c], "sel": sel_cores[c],
            "idxc1": idxc_cores[1][c], "selc1": selc_cores[1][c],
            "idxc2": idxc_cores[2][c], "selc2": selc_cores[2][c],
            "smat": smat_cores[c],
        })
    return in_maps, CNT, CNTC, node_order, apply_b1, apply_b2, apply_b3


_NC_CACHE = {}


def kernel_with_results(x, W1, b1, W2, b2, W3, b3, edge_index, trace=False):
    (in_maps, CNT, CNTC, node_order, apply_b1, apply_b2,
     apply_b3) = _make_in_maps(x, W1, b1, W2, b2, W3, b3, edge_index)
    key = (tuple(CNT), tuple(CNTC[1]), tuple(CNTC[2]),
           apply_b1, apply_b2, apply_b3)
    if key not in _NC_CACHE:
        _NC_CACHE[key] = _build_nc(CNT, CNTC, apply_b1, apply_b2, apply_b3)
    nc = _NC_CACHE[key]
    res = run_bass_kernel_spmd(
        nc, in_maps, core_ids=list(range(N_CORES)), trace=trace)
    rows = np.concatenate(
        [np.asarray(res.results[c]["out"]) for c in range(N_CORES)], axis=0)
    full = np.zeros((N_NODES, rows.shape[1]), np.float32)
    real = node_order >= 0
    full[node_order[real]] = rows[real]
    return full, res


def kernel(x, W1, b1, W2, b2, W3, b3, edge_index):
    full, _ = kernel_with_results(x, W1, b1, W2, b2, W3, b3, edge_index)
    return full
